# revision 31
# baseline (speedup 1.0000x reference)
"""Multi-head attention (B=2, S=2048, D=1024, H=16) on 8 Trainium2 NeuronCores.

Sharding: core c -> (batch b = c//4, head-group g = c%4).  Each core computes
Q/K/V projections for its 4 heads (256 features), causal attention for those
heads over the full sequence, and a partial O-projection (its 256 attn
features x full Wo.T slice).  The host sums the 4 partial outputs per batch
and folds in the biases that commute with the reduction (bo, bv @ Wo.T).

v2 structure (single fused pipeline, PE-dense):
  - Projections are split into per-chunk units (K/Q per 512-token chunk,
    V per 128-token tile, O per 128x512 block) and woven into the attention
    stream by a debt-based scheduler: the ScalarE exp stream (~82us, the
    attention-phase bottleneck) runs concurrently with projection matmuls
    instead of in a separate phase.
  - QK for the two heads of a feature tile are emitted adjacently; their
    lhsT base partitions (0/64) auto-derive row-tile positions T0/T8, so the
    two 64-contraction matmuls run concurrently in the PE array halves.
  - V is augmented with 64 columns of ones -> the PV matmul's output rows
    64..127 hold the softmax denominator replicated 64x, so normalization is
    two DVE ops (reciprocal_approx_fast + multiply); no partition broadcast.
  - exp on ScalarE with fused 1/sqrt(dk) scale; no max-subtraction (scores
    are O(5) for this data, exp is exact to 2 ULP, f32 cannot overflow).
  - masking: multiplicative bf16 tiles after exp, duplicated per head pair;
    partially-masked tiles carry a start column c0 so QK/exp/PV skip the
    dead q-range.
  - outputs are written bf16 (partials are summed f32 on host).
"""

import hashlib
from collections import deque
from contextlib import ExitStack

import ml_dtypes
import numpy as np

import concourse.bass as bass  # noqa: F401  (AP helpers)
import concourse.tile as tile
from concourse import bacc, mybir
from concourse.bass_utils import run_bass_kernel_spmd

B, S, D, H = 2, 2048, 1024, 16
DK = D // H                  # 64 head dim
NCORE = 8
GROUPS = NCORE // B          # 4 head-groups per batch
HPC = H // GROUPS            # 4 heads per core
FPC = HPC * DK               # 256 features per core
FT = FPC // 128              # 2 feature tiles per core
DT = D // 128                # 8 d_in tiles
TT = S // 128                # 16 token tiles (k tiles)
QB = 512                     # query block (free-dim) size in attention
NQB = S // QB                # 4 query blocks
NCH = 512                    # psum free-dim chunk for projections
XCH = 512                    # input stream DMA column chunk
VW = 2 * DK                  # augmented V width: [V | ones]
BF = mybir.dt.bfloat16
F32 = mybir.dt.float32
BFNP = ml_dtypes.bfloat16

# module-level knobs for test.py
PROFILE = False
TRACE_CORES = None
LAST_RESULT = None
DEBUG_TAPS = False

_program_cache: dict = {}


def _classify_mask(mask2d: np.ndarray):
    """Classify (S, S) keep-mask into per-(qblock, ktile) modes.

    Returns (plan, patterns): plan[qb] is a list of (kt, mask_id|None, c0, c1)
    for tiles that are at least partially kept, where c0 is the first
    q-column (within the block) with any kept key and [c0, c1) the strip
    needing the multiplicative mask; patterns is a list of [128, 2*w] bf16
    tiles (k on partitions, the mask strip duplicated for the head pair).
    """
    keep = np.asarray(mask2d) != 0
    patterns = []
    pattern_ids = {}
    plan = []
    for qb in range(NQB):
        row = []
        for kt in range(TT):
            blk = keep[qb * QB:(qb + 1) * QB, kt * 128:(kt + 1) * 128].T
            if not blk.any():
                continue
            if blk.all():
                row.append((kt, None, 0, 0))
                continue
            anyk = blk.any(axis=0)
            allk = blk.all(axis=0)
            c0 = int(np.flatnonzero(anyk)[0])
            notall = np.flatnonzero(~allk)
            c1 = int(notall[-1]) + 1 if notall.size else c0
            pat = blk[:, c0:c1]
            key = pat.tobytes()
            mid = pattern_ids.get(key)
            if mid is None:
                mid = len(patterns)
                pattern_ids[key] = mid
                # duplicate for the two heads sharing one [128, 2, w] mul
                patterns.append(np.concatenate([pat, pat], axis=1).astype(BFNP))
            row.append((kt, mid, c0, c1))
        plan.append(row)
    return plan, patterns


def build_program(plan, npat, pw):
    nc = bacc.Bacc("TRN2", target_bir_lowering=False, debug=False,
                   num_devices=NCORE)
    qT = nc.dram_tensor("qT", (D, S), BF, kind="ExternalInput").ap()
    kT = nc.dram_tensor("kT", (D, S), BF, kind="ExternalInput").ap()
    vT = nc.dram_tensor("vT", (D, S), BF, kind="ExternalInput").ap()
    wqT = nc.dram_tensor("wqT", (D, FPC), BF, kind="ExternalInput").ap()
    wkT = nc.dram_tensor("wkT", (D, FPC), BF, kind="ExternalInput").ap()
    wvT = nc.dram_tensor("wvT", (D, FPC), BF, kind="ExternalInput").ap()
    woT = nc.dram_tensor("woT", (FPC, D), BF, kind="ExternalInput").ap()
    bqk = nc.dram_tensor("bqk", (2, FT, 128), F32, kind="ExternalInput").ap()
    masks = None
    if npat:
        masks = nc.dram_tensor("masks", (npat, 128, pw), BF,
                               kind="ExternalInput").ap()
    out = nc.dram_tensor("out", (S, D), BF, kind="ExternalOutput").ap()
    taps = None
    if DEBUG_TAPS:
        taps = {
            n: nc.dram_tensor(f"dbg_{n}", shape, BF,
                              kind="ExternalOutput").ap()
            for n, shape in (("q", (128, FT, S)), ("k", (128, FT, S)),
                             ("attn", (128, FT, S)),
                             ("v", (128, TT, HPC, VW)))
        }

    with tile.TileContext(nc) as tc, ExitStack() as ctx:
        singles = ctx.enter_context(tc.tile_pool(name="singles", bufs=1))
        ppool = ctx.enter_context(tc.tile_pool(name="ppool", bufs=4))
        npool = ctx.enter_context(tc.tile_pool(name="npool", bufs=4))
        opool = ctx.enter_context(tc.tile_pool(name="opool", bufs=4))
        psacc = ctx.enter_context(tc.tile_pool(name="psacc", bufs=2, space="PSUM"))
        psS = ctx.enter_context(tc.tile_pool(name="psS", bufs=2, space="PSUM"))
        psPV = ctx.enter_context(tc.tile_pool(name="psPV", bufs=1, space="PSUM"))

        # ---- SBUF residents ----
        wq_sb = singles.tile([128, DT, FPC], BF)
        wk_sb = singles.tile([128, DT, FPC], BF)
        wv_sb = singles.tile([128, DT, FPC], BF)
        wo_sb = singles.tile([128, FT, D], BF)
        bias_sb = singles.tile([128, 2, FT], F32)
        mask_sb = None
        if npat:
            mask_sb = singles.tile([128, npat, pw], BF, name="mask_sb")
        q_sb = singles.tile([128, FT, S], BF)
        # K^T stored twice, block-diagonal: kz[:, 0] has the even head's
        # features on partitions 0..63 and ZEROS on 64..127, kz[:, 1] the
        # odd head's on 64..127 with zeros on 0..63.  QK then contracts
        # over all 128 partitions in the default 128x128 array mode (the
        # zero half annihilates the other head), so no PE tile-mode
        # switches happen anywhere in the program.
        kz_sb = singles.tile([128, 2, FT, S], BF)
        attn_sb = singles.tile([128, FT, S], BF)
        v_sb = singles.tile([128, TT, HPC, VW], BF)
        xv_sb = singles.tile([128, DT, S], BF)
        xk_sb = singles.tile([128, DT, S], BF)
        xq_sb = singles.tile([128, DT, S], BF)

        # ---- DMA issue order: first-need first ----
        def stream_chunk(x_sb, x_dram, ch, split=False):
            xr = x_dram.rearrange("(t p) f -> p t f", p=128)
            if split:
                # per-d_in-tile descriptors so the first projection's
                # dt-loop can start before the whole chunk lands
                for dt in range(DT):
                    nc.sync.dma_start(
                        x_sb[:, dt, ch * XCH:(ch + 1) * XCH],
                        xr[:, dt, ch * XCH:(ch + 1) * XCH])
            else:
                nc.sync.dma_start(
                    x_sb[:, :, ch * XCH:(ch + 1) * XCH],
                    xr[:, :, ch * XCH:(ch + 1) * XCH])

        nc.sync.dma_start(wk_sb, wkT.rearrange("(t p) f -> p t f", p=128))
        stream_chunk(xk_sb, kT, 0, split=True)
        nc.sync.dma_start(bias_sb, bqk.rearrange("a b p -> p a b"))
        if npat:
            nc.sync.dma_start(mask_sb, masks.rearrange("m p f -> p m f"))
        nc.sync.dma_start(wq_sb, wqT.rearrange("(t p) f -> p t f", p=128))
        stream_chunk(xq_sb, qT, 0)
        nc.sync.dma_start(wv_sb, wvT.rearrange("(t p) f -> p t f", p=128))
        stream_chunk(xv_sb, vT, 0)
        nc.sync.dma_start(wo_sb, woT.rearrange("(t p) f -> p t f", p=128))
        for ch in range(1, S // XCH):
            stream_chunk(xk_sb, kT, ch)
            stream_chunk(xq_sb, qT, ch)
            stream_chunk(xv_sb, vT, ch)

        # ones columns of the augmented V (denominator accumulators).
        # Layout [ones | V]: the PV output then carries the denominator on
        # PSUM partitions 0..63 (reciprocal_approx_fast only works at base
        # partition 0) and attnU on partitions 64..127.
        for tt in range(TT):
            nc.vector.memset(v_sb[:, tt, :, 0:DK], 1.0)
        # zero halves of the block-diagonal K store
        nc.vector.memset(kz_sb[64:128, 0, :, :], 0.0)
        nc.vector.memset(kz_sb[0:64, 1, :, :], 0.0)

        # ---- projection / output units (PE work the weave can draw on) ----
        # Two PSUM accumulation chains are interleaved matmul-by-matmul
        # (ping-pong between the two psacc banks): consecutive accumulating
        # matmuls into the SAME bank serialize at full (398+N)/2.4 latency
        # because the drain cannot overlap a same-bank fill, costing ~45%;
        # alternating banks restores the N/2.4 streaming rate.  `held`
        # carries the two open accumulation tiles across quarter-units so
        # the weave can still interleave attention work between them.
        held = {}

        def kq_quarter(which, nch, part):
            x_sb, w_sb = (xk_sb, wk_sb) if which == "k" else (xq_sb, wq_sb)
            if part == 0:
                for ft in range(FT):
                    held[(which, nch, ft)] = psacc.tile(
                        [128, NCH], F32, tag="acc", name="acc")
            for dt in (2 * part, 2 * part + 1):
                for ft in range(FT):
                    nc.tensor.matmul(
                        held[(which, nch, ft)],
                        lhsT=w_sb[:, dt, ft * 128:(ft + 1) * 128],
                        rhs=x_sb[:, dt, nch * NCH:(nch + 1) * NCH],
                        start=(dt == 0), stop=(dt == DT - 1))
            if part == 3:
                cols = slice(nch * NCH, (nch + 1) * NCH)
                for ft in range(FT):
                    ps = held.pop((which, nch, ft))
                    if which == "q":
                        nc.vector.tensor_scalar_add(
                            q_sb[:, ft, cols], ps, bias_sb[:, 0, ft:ft + 1])
                    else:
                        nc.vector.tensor_scalar_add(
                            kz_sb[0:64, 0, ft, cols], ps[0:64, :],
                            bias_sb[0:64, 1, ft:ft + 1])
                        nc.vector.tensor_scalar_add(
                            kz_sb[64:128, 1, ft, cols], ps[64:128, :],
                            bias_sb[64:128, 1, ft:ft + 1])

        def v_quarter(ta, part):
            pair = (ta, ta + 1)
            if part == 0:
                for tt in pair:
                    held[("v", tt)] = psacc.tile([128, FPC], F32,
                                                 tag="acc", name="acc")
            for dt in (2 * part, 2 * part + 1):
                for tt in pair:
                    nc.tensor.matmul(
                        held[("v", tt)],
                        lhsT=xv_sb[:, dt, tt * 128:(tt + 1) * 128],
                        rhs=wv_sb[:, dt, :],
                        start=(dt == 0), stop=(dt == DT - 1))
            if part == 3:
                for tt in pair:
                    ps = held.pop(("v", tt))
                    nc.vector.tensor_copy(
                        v_sb[:, tt, :, DK:VW],
                        ps.rearrange("p (h d) -> p h d", h=HPC))

        def o_half(qt, hd):
            if hd == 0:
                for nch in range(D // NCH):
                    held[("o", qt, nch)] = psacc.tile(
                        [128, NCH], F32, tag="acc", name="acc")
            for nch in range(D // NCH):
                nc.tensor.matmul(
                    held[("o", qt, nch)],
                    lhsT=attn_sb[:, hd, qt * 128:(qt + 1) * 128],
                    rhs=wo_sb[:, hd, nch * NCH:(nch + 1) * NCH],
                    start=(hd == 0), stop=(hd == FT - 1))
            if hd == FT - 1:
                for nch in range(D // NCH):
                    ps = held.pop(("o", qt, nch))
                    ob = opool.tile([128, NCH], BF)
                    nc.vector.tensor_copy(ob, ps)
                    nc.sync.dma_start(
                        out[qt * 128:(qt + 1) * 128,
                            nch * NCH:(nch + 1) * NCH], ob)

        # filler: (pe_ns, emit_fn, gate) — gate = phase index 2*qb+hp by
        # which the unit MUST have been emitted (its results feed that
        # phase's attention).
        KQ_NS = 4 * NCH / 2.4
        V_NS = 4 * FPC / 2.4
        O_NS = 2 * NCH / 2.4
        filler = deque()

        def push_proj(qb):
            for which in ("k", "q"):
                for part in range(4):
                    filler.append(
                        (KQ_NS,
                         lambda w=which, p=part: kq_quarter(w, qb, p),
                         2 * qb))
            for ta in (4 * qb, 4 * qb + 2):
                for part in range(4):
                    filler.append(
                        (V_NS, lambda t=ta, p=part: v_quarter(t, p), 2 * qb))

        def push_o(qb):
            for qt in range(qb * QB // 128, (qb + 1) * QB // 128):
                for hd in range(FT):
                    filler.append(
                        (O_NS, lambda q=qt, h=hd: o_half(q, h), 2 * qb + 4))

        state = {"debt": 0.0}

        def drain():
            while state["debt"] > 0 and filler:
                pe_ns, fn, gate = filler.popleft()
                fn()
                state["debt"] -= pe_ns

        def force(phase):
            while filler and filler[0][2] <= phase:
                pe_ns, fn, gate = filler.popleft()
                fn()

        # ---- prelude: minimum needed to start (qb0, hp0) attention ----
        for part in range(4):
            kq_quarter("k", 0, part)
        for part in range(4):
            kq_quarter("q", 0, part)
        for part in range(4):
            v_quarter(0, part)
        for part in range(4):
            filler.append((V_NS, lambda p=part: v_quarter(2, p), 1))

        # ---- fused attention pipeline ----
        inv_sqrt_dk = float(1.0 / np.sqrt(DK))
        for qb in range(NQB):
            if qb + 1 < NQB:
                push_proj(qb + 1)
            kts = plan[qb]
            if not kts:
                push_o(qb)
                continue
            n = len(kts)
            for hp in range(HPC // 2):
                force(2 * qb + hp)
                pv = [psPV.tile([128, QB], F32, tag=f"pv{j}", name=f"pv{j}")
                      for j in (0, 1)]

                def pv_mms(i, kt, c0, pT):
                    for j in (0, 1):
                        h = 2 * hp + j
                        nc.tensor.matmul(pv[j][:, c0:],
                                         lhsT=v_sb[:, kt, h, :],
                                         rhs=pT[:, j, c0:],
                                         start=(i == 0), stop=(i == n - 1))
                    state["debt"] -= 2 * (QB - c0) / 2.4

                def qk_mms(kt, c0):
                    s_ps = psS.tile([128, 2, QB], F32, name="s_ps")
                    for j in (0, 1):
                        nc.tensor.matmul(
                            s_ps[:, j, c0:],
                            lhsT=kz_sb[:, j, hp, kt * 128:(kt + 1) * 128],
                            rhs=q_sb[:, hp, qb * QB + c0:(qb + 1) * QB],
                            start=True, stop=True)
                    state["debt"] -= (QB - c0) / 2.4
                    return s_ps

                # software pipeline: the exp for tile i is followed
                # immediately (in PE program order) by the QK of tile i+1,
                # so the ScalarE exp stream always has its next input ready;
                # filler and the PV of tile i-1 then absorb the remaining
                # exp window.
                prev = None
                s_cur = qk_mms(kts[0][0], kts[0][2])
                for i, (kt, mid, c0, c1) in enumerate(kts):
                    pT = ppool.tile([128, 2, QB], BF, tag="pt")
                    nc.scalar.activation(pT[:, :, c0:], s_cur[:, :, c0:],
                                         mybir.ActivationFunctionType.Exp,
                                         scale=inv_sqrt_dk)
                    if mid is not None and c1 > c0:
                        assert mask_sb is not None
                        w = c1 - c0
                        nc.vector.tensor_mul(
                            pT[:, :, c0:c1], pT[:, :, c0:c1],
                            mask_sb[:, mid, 0:2 * w].rearrange(
                                "p (j w) -> p j w", j=2))
                    if i + 1 < n:
                        s_cur = qk_mms(kts[i + 1][0], kts[i + 1][2])
                    state["debt"] += (2 * (QB - c0) + 360) / 1.2 + 250
                    drain()
                    if prev is not None:
                        pv_mms(*prev)
                    prev = (i, kt, c0, pT)
                if prev is not None:
                    pv_mms(*prev)
                # normalize: attn^T[d, q] = attnU^T[d, q] * (1/denom[q])
                for j in (0, 1):
                    pr = j * 64
                    rec = npool.tile([64, QB], F32, tag="rec")
                    nc.vector.reciprocal_approx_fast(rec, pv[j][0:DK, :])
                    nc.vector.tensor_mul(
                        attn_sb[pr:pr + DK, hp, qb * QB:(qb + 1) * QB],
                        pv[j][DK:VW, :], rec)
                # pull filler over the DVE normalize chain so the next
                # head-pair's first PV doesn't idle on the psPV banks
                state["debt"] += 1400
                drain()
            push_o(qb)

        # drain what's left (the last O-projection blocks mostly)
        while filler:
            pe_ns, fn, gate = filler.popleft()
            fn()

        if taps is not None:
            nc.sync.dma_start(taps["q"], q_sb)
            nc.sync.dma_start(taps["k"], kz_sb[:, 0, :, :])
            nc.sync.dma_start(taps["attn"], attn_sb)
            nc.sync.dma_start(taps["v"], v_sb)

    nc.compile()
    return nc


def _get_program(mask2d: np.ndarray):
    key = (hashlib.sha1(np.ascontiguousarray(mask2d).tobytes()).hexdigest()
           + ("+taps" if DEBUG_TAPS else ""))
    hit = _program_cache.get(key)
    if hit is not None:
        return hit
    plan, patterns = _classify_mask(mask2d)
    pw = max((p.shape[1] for p in patterns), default=0)
    nc = build_program(plan, len(patterns), pw)
    if patterns:
        pat = np.zeros((len(patterns), 128, pw), BFNP)
        for i, p in enumerate(patterns):
            pat[i, :, :p.shape[1]] = p
    else:
        pat = None
    _program_cache[key] = (nc, pat)
    return nc, pat


def kernel(**inputs) -> np.ndarray:
    global LAST_RESULT
    query = np.asarray(inputs["query"], np.float32)
    key = np.asarray(inputs["key"], np.float32)
    value = np.asarray(inputs["value"], np.float32)
    mask = np.asarray(inputs["mask"])
    Wq = np.asarray(inputs["Wq"], np.float32)
    bq = np.asarray(inputs["bq"], np.float32)
    Wk = np.asarray(inputs["Wk"], np.float32)
    bk = np.asarray(inputs["bk"], np.float32)
    Wv = np.asarray(inputs["Wv"], np.float32)
    bv = np.asarray(inputs["bv"], np.float32)
    Wo = np.asarray(inputs["Wo"], np.float32)
    bo = np.asarray(inputs["bo"], np.float32)

    nc, pat = _get_program(mask.reshape(S, S))

    WqT, WkT, WvT, WoT = Wq.T, Wk.T, Wv.T, Wo.T
    xT = {
        t: [np.ascontiguousarray(x[b].T).astype(BFNP) for b in range(B)]
        for t, x in (("qT", query), ("kT", key), ("vT", value))
    }
    in_maps = []
    for c in range(NCORE):
        b, g = divmod(c, GROUPS)
        f0 = g * FPC
        m = {
            "qT": xT["qT"][b],
            "kT": xT["kT"][b],
            "vT": xT["vT"][b],
            "wqT": np.ascontiguousarray(WqT[:, f0:f0 + FPC]).astype(BFNP),
            "wkT": np.ascontiguousarray(WkT[:, f0:f0 + FPC]).astype(BFNP),
            "wvT": np.ascontiguousarray(WvT[:, f0:f0 + FPC]).astype(BFNP),
            "woT": np.ascontiguousarray(WoT[f0:f0 + FPC, :]).astype(BFNP),
            "bqk": np.stack([bq[f0:f0 + FPC].reshape(FT, 128),
                             bk[f0:f0 + FPC].reshape(FT, 128)]).astype(np.float32),
        }
        if pat is not None:
            m["masks"] = pat
        in_maps.append(m)

    res = run_bass_kernel_spmd(
        nc, in_maps, core_ids=list(range(NCORE)),
        trace=PROFILE,
        trace_cores=(TRACE_CORES if TRACE_CORES is not None
                     else (list(range(NCORE)) if PROFILE else None)),
    )
    LAST_RESULT = res

    host_bias = bo + bv @ WoT  # (D,) folded V/O biases, added once per batch
    out = np.empty((B, S, D), np.float32)
    for b in range(B):
        acc = res.results[b * GROUPS]["out"].astype(np.float32)
        for g in range(1, GROUPS):
            acc = acc + res.results[b * GROUPS + g]["out"].astype(np.float32)
        out[b] = acc + host_bias
    return out


# revision 38
# speedup vs baseline: 1.0228x; 1.0228x over previous
"""Multi-head attention (B=2, S=2048, D=1024, H=16) on 8 Trainium2 NeuronCores.

Sharding: core c -> (batch b = c//4, head-group g = c%4).  Each core computes
Q/K/V projections for its 4 heads (256 features), causal attention for those
heads over the full sequence, and a partial O-projection (its 256 attn
features x full Wo.T slice).  The host sums the 4 partial outputs per batch
and folds in the biases that commute with the reduction (bo, bv @ Wo.T).

v2 structure (single fused pipeline, PE-dense):
  - Projections are split into per-chunk units (K/Q per 512-token chunk,
    V per 128-token tile, O per 128x512 block) and woven into the attention
    stream by a debt-based scheduler: the ScalarE exp stream (~82us, the
    attention-phase bottleneck) runs concurrently with projection matmuls
    instead of in a separate phase.
  - QK for the two heads of a feature tile are emitted adjacently; their
    lhsT base partitions (0/64) auto-derive row-tile positions T0/T8, so the
    two 64-contraction matmuls run concurrently in the PE array halves.
  - V is augmented with 64 columns of ones -> the PV matmul's output rows
    64..127 hold the softmax denominator replicated 64x, so normalization is
    two DVE ops (reciprocal_approx_fast + multiply); no partition broadcast.
  - exp on ScalarE with fused 1/sqrt(dk) scale; no max-subtraction (scores
    are O(5) for this data, exp is exact to 2 ULP, f32 cannot overflow).
  - masking: multiplicative bf16 tiles after exp, duplicated per head pair;
    partially-masked tiles carry a start column c0 so QK/exp/PV skip the
    dead q-range.
  - outputs are written bf16 (partials are summed f32 on host).
"""

import hashlib
from collections import deque
from contextlib import ExitStack

import ml_dtypes
import numpy as np

import concourse.bass as bass  # noqa: F401  (AP helpers)
import concourse.tile as tile
from concourse import bacc, mybir
from concourse.bass_utils import run_bass_kernel_spmd

B, S, D, H = 2, 2048, 1024, 16
DK = D // H                  # 64 head dim
NCORE = 8
GROUPS = NCORE // B          # 4 head-groups per batch
HPC = H // GROUPS            # 4 heads per core
FPC = HPC * DK               # 256 features per core
FT = FPC // 128              # 2 feature tiles per core
DT = D // 128                # 8 d_in tiles
TT = S // 128                # 16 token tiles (k tiles)
QB = 512                     # query block (free-dim) size in attention
NQB = S // QB                # 4 query blocks
NCH = 512                    # psum free-dim chunk for projections
XCH = 512                    # input stream DMA column chunk
VW = 2 * DK                  # augmented V width: [V | ones]
BF = mybir.dt.bfloat16
F32 = mybir.dt.float32
BFNP = ml_dtypes.bfloat16

# module-level knobs for test.py
PROFILE = False
TRACE_CORES = None
LAST_RESULT = None
DEBUG_TAPS = False

_program_cache: dict = {}


def _classify_mask(mask2d: np.ndarray):
    """Classify (S, S) keep-mask into per-(qblock, ktile) modes.

    Returns (plan, patterns): plan[qb] is a list of (kt, mask_id|None, c0, c1)
    for tiles that are at least partially kept, where c0 is the first
    q-column (within the block) with any kept key and [c0, c1) the strip
    needing the multiplicative mask; patterns is a list of [128, 2*w] bf16
    tiles (k on partitions, the mask strip duplicated for the head pair).
    """
    keep = np.asarray(mask2d) != 0
    patterns = []
    pattern_ids = {}
    plan = []
    for qb in range(NQB):
        row = []
        for kt in range(TT):
            blk = keep[qb * QB:(qb + 1) * QB, kt * 128:(kt + 1) * 128].T
            if not blk.any():
                continue
            if blk.all():
                row.append((kt, None, 0, 0))
                continue
            anyk = blk.any(axis=0)
            allk = blk.all(axis=0)
            c0 = int(np.flatnonzero(anyk)[0])
            notall = np.flatnonzero(~allk)
            c1 = int(notall[-1]) + 1 if notall.size else c0
            pat = blk[:, c0:c1]
            key = pat.tobytes()
            mid = pattern_ids.get(key)
            if mid is None:
                mid = len(patterns)
                pattern_ids[key] = mid
                # duplicate for the two heads sharing one [128, 2, w] mul
                patterns.append(np.concatenate([pat, pat], axis=1).astype(BFNP))
            row.append((kt, mid, c0, c1))
        plan.append(row)
    return plan, patterns


def build_program(plan, npat, pw):
    nc = bacc.Bacc("TRN2", target_bir_lowering=False, debug=False,
                   num_devices=NCORE)
    qT = nc.dram_tensor("qT", (D, S), BF, kind="ExternalInput").ap()
    kT = nc.dram_tensor("kT", (D, S), BF, kind="ExternalInput").ap()
    vT = nc.dram_tensor("vT", (D, S), BF, kind="ExternalInput").ap()
    wqT = nc.dram_tensor("wqT", (D, FPC), BF, kind="ExternalInput").ap()
    wkT = nc.dram_tensor("wkT", (D, FPC), BF, kind="ExternalInput").ap()
    wvT = nc.dram_tensor("wvT", (D, FPC), BF, kind="ExternalInput").ap()
    woT = nc.dram_tensor("woT", (FPC, D), BF, kind="ExternalInput").ap()
    bqk = nc.dram_tensor("bqk", (2, FT, 128), F32, kind="ExternalInput").ap()
    masks = None
    if npat:
        masks = nc.dram_tensor("masks", (npat, 128, pw), BF,
                               kind="ExternalInput").ap()
    out = nc.dram_tensor("out", (S, D), BF, kind="ExternalOutput").ap()
    taps = None
    if DEBUG_TAPS:
        taps = {
            n: nc.dram_tensor(f"dbg_{n}", shape, BF,
                              kind="ExternalOutput").ap()
            for n, shape in (("q", (128, FT, S)), ("k", (128, FT, S)),
                             ("attn", (128, FT, S)),
                             ("v", (128, TT, HPC, VW)))
        }

    with tile.TileContext(nc) as tc, ExitStack() as ctx:
        singles = ctx.enter_context(tc.tile_pool(name="singles", bufs=1))
        ppool = ctx.enter_context(tc.tile_pool(name="ppool", bufs=4))
        npool = ctx.enter_context(tc.tile_pool(name="npool", bufs=4))
        opool = ctx.enter_context(tc.tile_pool(name="opool", bufs=4))
        psacc = ctx.enter_context(tc.tile_pool(name="psacc", bufs=2, space="PSUM"))
        psS = ctx.enter_context(tc.tile_pool(name="psS", bufs=2, space="PSUM"))
        psPV = ctx.enter_context(tc.tile_pool(name="psPV", bufs=1, space="PSUM"))

        # ---- SBUF residents ----
        wq_sb = singles.tile([128, DT, FPC], BF)
        wk_sb = singles.tile([128, DT, FPC], BF)
        wv_sb = singles.tile([128, DT, FPC], BF)
        wo_sb = singles.tile([128, FT, D], BF)
        bias_sb = singles.tile([128, 2, FT], F32)
        mask_sb = None
        if npat:
            mask_sb = singles.tile([128, npat, pw], BF, name="mask_sb")
        q_sb = singles.tile([128, FT, S], BF)
        k_sb = singles.tile([128, FT, S], BF)
        attn_sb = singles.tile([128, FT, S], BF)
        v_sb = singles.tile([128, TT, HPC, VW], BF)
        xv_sb = singles.tile([128, DT, S], BF)
        xk_sb = singles.tile([128, DT, S], BF)
        xq_sb = singles.tile([128, DT, S], BF)

        # ---- DMA issue order: first-need first ----
        def stream_chunk(x_sb, x_dram, ch, split=False):
            xr = x_dram.rearrange("(t p) f -> p t f", p=128)
            if split:
                # per-d_in-tile descriptors so the first projection's
                # dt-loop can start before the whole chunk lands
                for dt in range(DT):
                    nc.sync.dma_start(
                        x_sb[:, dt, ch * XCH:(ch + 1) * XCH],
                        xr[:, dt, ch * XCH:(ch + 1) * XCH])
            else:
                nc.sync.dma_start(
                    x_sb[:, :, ch * XCH:(ch + 1) * XCH],
                    xr[:, :, ch * XCH:(ch + 1) * XCH])

        nc.sync.dma_start(wk_sb, wkT.rearrange("(t p) f -> p t f", p=128))
        stream_chunk(xk_sb, kT, 0, split=True)
        nc.sync.dma_start(bias_sb, bqk.rearrange("a b p -> p a b"))
        if npat:
            nc.sync.dma_start(mask_sb, masks.rearrange("m p f -> p m f"))
        nc.sync.dma_start(wq_sb, wqT.rearrange("(t p) f -> p t f", p=128))
        stream_chunk(xq_sb, qT, 0)
        nc.sync.dma_start(wv_sb, wvT.rearrange("(t p) f -> p t f", p=128))
        stream_chunk(xv_sb, vT, 0)
        nc.sync.dma_start(wo_sb, woT.rearrange("(t p) f -> p t f", p=128))
        for ch in range(1, S // XCH):
            stream_chunk(xk_sb, kT, ch)
            stream_chunk(xq_sb, qT, ch)
            stream_chunk(xv_sb, vT, ch)

        # ones columns of the augmented V (denominator accumulators).
        # Layout [ones | V]: the PV output then carries the denominator on
        # PSUM partitions 0..63 (reciprocal_approx_fast only works at base
        # partition 0) and attnU on partitions 64..127.
        for tt in range(TT):
            nc.vector.memset(v_sb[:, tt, :, 0:DK], 1.0)

        # ---- projection / output units (PE work the weave can draw on) ----
        # K/Q/V units are split into two half-groups (4 d_in tiles each) so
        # the weave can interleave them at sub-microsecond granularity; the
        # two halves share one PSUM accumulation tile via `held`.
        held = {}

        def kq_half(which, nch, ft, half):
            x_sb, w_sb = (xk_sb, wk_sb) if which == "k" else (xq_sb, wq_sb)
            if half == 0:
                held[(which, nch, ft)] = psacc.tile([128, NCH], F32,
                                                    tag="acc", name="acc")
            ps = held[(which, nch, ft)]
            for dt in range(4 * half, 4 * half + 4):
                nc.tensor.matmul(
                    ps,
                    lhsT=w_sb[:, dt, ft * 128:(ft + 1) * 128],
                    rhs=x_sb[:, dt, nch * NCH:(nch + 1) * NCH],
                    start=(dt == 0), stop=(dt == DT - 1))
            if half == 1:
                del held[(which, nch, ft)]
                cols = slice(nch * NCH, (nch + 1) * NCH)
                if which == "q":
                    nc.vector.tensor_scalar_add(
                        q_sb[:, ft, cols], ps, bias_sb[:, 0, ft:ft + 1])
                else:
                    nc.vector.tensor_scalar_add(
                        k_sb[:, ft, cols], ps, bias_sb[:, 1, ft:ft + 1])

        def v_half(tt, half):
            if half == 0:
                held[("v", tt)] = psacc.tile([128, FPC], F32,
                                             tag="acc", name="acc")
            ps = held[("v", tt)]
            for dt in range(4 * half, 4 * half + 4):
                nc.tensor.matmul(ps,
                                 lhsT=xv_sb[:, dt, tt * 128:(tt + 1) * 128],
                                 rhs=wv_sb[:, dt, :],
                                 start=(dt == 0), stop=(dt == DT - 1))
            if half == 1:
                del held[("v", tt)]
                nc.vector.tensor_copy(v_sb[:, tt, :, DK:VW],
                                      ps.rearrange("p (h d) -> p h d", h=HPC))

        def o_unit(qt, nch):
            ps = psacc.tile([128, NCH], F32, tag="acc")
            for hd in range(FT):
                nc.tensor.matmul(
                    ps,
                    lhsT=attn_sb[:, hd, qt * 128:(qt + 1) * 128],
                    rhs=wo_sb[:, hd, nch * NCH:(nch + 1) * NCH],
                    start=(hd == 0), stop=(hd == FT - 1))
            ob = opool.tile([128, NCH], BF)
            nc.vector.tensor_copy(ob, ps)
            nc.sync.dma_start(
                out[qt * 128:(qt + 1) * 128, nch * NCH:(nch + 1) * NCH], ob)

        # filler: (pe_ns, emit_fn, gate) — gate = phase index 2*qb+hp by
        # which the unit MUST have been emitted (its results feed that
        # phase's attention).
        KQ_NS = 4 * NCH / 2.4
        V_NS = 4 * FPC / 2.4
        O_NS = 2 * NCH / 2.4
        filler = deque()

        def push_proj(qb):
            for ft in range(FT):
                for which in ("k", "q"):
                    for half in (0, 1):
                        filler.append(
                            (KQ_NS,
                             lambda w=which, f=ft, h=half: kq_half(w, qb, f, h),
                             2 * qb + ft))
            for tt in range(4 * qb, 4 * qb + 4):
                for half in (0, 1):
                    filler.append(
                        (V_NS, lambda t=tt, h=half: v_half(t, h), 2 * qb))

        def push_o(qb):
            for qt in range(qb * QB // 128, (qb + 1) * QB // 128):
                for nch in range(D // NCH):
                    filler.append(
                        (O_NS, lambda q=qt, n=nch: o_unit(q, n), 2 * qb + 4))

        state = {"debt": 0.0}

        def drain():
            while state["debt"] > 0 and filler:
                pe_ns, fn, gate = filler.popleft()
                fn()
                state["debt"] -= pe_ns

        def force(phase):
            while filler and filler[0][2] <= phase:
                pe_ns, fn, gate = filler.popleft()
                fn()

        # ---- prelude: minimum needed to start (qb0, hp0) attention ----
        for half in (0, 1):
            kq_half("k", 0, 0, half)
        for half in (0, 1):
            kq_half("q", 0, 0, half)
        for tt in (0, 1):
            for half in (0, 1):
                v_half(tt, half)
        for tt in (2, 3):
            for half in (0, 1):
                filler.append((V_NS, lambda t=tt, h=half: v_half(t, h), 1))
        for which in ("k", "q"):
            for half in (0, 1):
                filler.append(
                    (KQ_NS, lambda w=which, h=half: kq_half(w, 0, 1, h), 1))

        # ---- fused attention pipeline ----
        inv_sqrt_dk = float(1.0 / np.sqrt(DK))
        for qb in range(NQB):
            if qb + 1 < NQB:
                push_proj(qb + 1)
            kts = plan[qb]
            if not kts:
                push_o(qb)
                continue
            n = len(kts)
            for hp in range(HPC // 2):
                force(2 * qb + hp)
                pv = [psPV.tile([128, QB], F32, tag=f"pv{j}", name=f"pv{j}")
                      for j in (0, 1)]

                def pv_mms(i, kt, c0, pT):
                    for j in (0, 1):
                        h = 2 * hp + j
                        nc.tensor.matmul(pv[j][:, c0:],
                                         lhsT=v_sb[:, kt, h, :],
                                         rhs=pT[:, j, c0:],
                                         start=(i == 0), stop=(i == n - 1))
                    state["debt"] -= 2 * (QB - c0) / 2.4

                def qk_mms(kt, c0):
                    s_ps = psS.tile([128, 2, QB], F32, name="s_ps")
                    for j in (0, 1):
                        pr = j * 64
                        nc.tensor.matmul(
                            s_ps[:, j, c0:],
                            lhsT=k_sb[pr:pr + DK, hp, kt * 128:(kt + 1) * 128],
                            rhs=q_sb[pr:pr + DK, hp,
                                     qb * QB + c0:(qb + 1) * QB],
                            start=True, stop=True)
                    state["debt"] -= (QB - c0) / 2.4
                    return s_ps

                # software pipeline: the exp for tile i is followed
                # immediately (in PE program order) by the QK of tile i+1,
                # so the ScalarE exp stream always has its next input ready;
                # filler and the PV of tile i-1 then absorb the remaining
                # exp window.
                prev = None
                s_cur = qk_mms(kts[0][0], kts[0][2])
                for i, (kt, mid, c0, c1) in enumerate(kts):
                    pT = ppool.tile([128, 2, QB], BF, tag="pt")
                    nc.scalar.activation(pT[:, :, c0:], s_cur[:, :, c0:],
                                         mybir.ActivationFunctionType.Exp,
                                         scale=inv_sqrt_dk)
                    if mid is not None and c1 > c0:
                        assert mask_sb is not None
                        w = c1 - c0
                        nc.vector.tensor_mul(
                            pT[:, :, c0:c1], pT[:, :, c0:c1],
                            mask_sb[:, mid, 0:2 * w].rearrange(
                                "p (j w) -> p j w", j=2))
                    if i + 1 < n:
                        s_cur = qk_mms(kts[i + 1][0], kts[i + 1][2])
                    state["debt"] += (2 * (QB - c0) + 360) / 1.2 + 250
                    drain()
                    if prev is not None:
                        pv_mms(*prev)
                    prev = (i, kt, c0, pT)
                if prev is not None:
                    pv_mms(*prev)
                # normalize: attn^T[d, q] = attnU^T[d, q] * (1/denom[q])
                for j in (0, 1):
                    pr = j * 64
                    rec = npool.tile([64, QB], F32, tag="rec")
                    nc.vector.reciprocal_approx_fast(rec, pv[j][0:DK, :])
                    nc.vector.tensor_mul(
                        attn_sb[pr:pr + DK, hp, qb * QB:(qb + 1) * QB],
                        pv[j][DK:VW, :], rec)
                # pull filler over the DVE normalize chain so the next
                # head-pair's first PV doesn't idle on the psPV banks
                state["debt"] += 1400
                drain()
            push_o(qb)

        # drain what's left (the last O-projection blocks mostly)
        while filler:
            pe_ns, fn, gate = filler.popleft()
            fn()

        if taps is not None:
            nc.sync.dma_start(taps["q"], q_sb)
            nc.sync.dma_start(taps["k"], k_sb)
            nc.sync.dma_start(taps["attn"], attn_sb)
            nc.sync.dma_start(taps["v"], v_sb)

    nc.compile()
    return nc


def _get_program(mask2d: np.ndarray):
    key = (hashlib.sha1(np.ascontiguousarray(mask2d).tobytes()).hexdigest()
           + ("+taps" if DEBUG_TAPS else ""))
    hit = _program_cache.get(key)
    if hit is not None:
        return hit
    plan, patterns = _classify_mask(mask2d)
    pw = max((p.shape[1] for p in patterns), default=0)
    nc = build_program(plan, len(patterns), pw)
    if patterns:
        pat = np.zeros((len(patterns), 128, pw), BFNP)
        for i, p in enumerate(patterns):
            pat[i, :, :p.shape[1]] = p
    else:
        pat = None
    _program_cache[key] = (nc, pat)
    return nc, pat


def kernel(**inputs) -> np.ndarray:
    global LAST_RESULT
    query = np.asarray(inputs["query"], np.float32)
    key = np.asarray(inputs["key"], np.float32)
    value = np.asarray(inputs["value"], np.float32)
    mask = np.asarray(inputs["mask"])
    Wq = np.asarray(inputs["Wq"], np.float32)
    bq = np.asarray(inputs["bq"], np.float32)
    Wk = np.asarray(inputs["Wk"], np.float32)
    bk = np.asarray(inputs["bk"], np.float32)
    Wv = np.asarray(inputs["Wv"], np.float32)
    bv = np.asarray(inputs["bv"], np.float32)
    Wo = np.asarray(inputs["Wo"], np.float32)
    bo = np.asarray(inputs["bo"], np.float32)

    nc, pat = _get_program(mask.reshape(S, S))

    WqT, WkT, WvT, WoT = Wq.T, Wk.T, Wv.T, Wo.T
    xT = {
        t: [np.ascontiguousarray(x[b].T).astype(BFNP) for b in range(B)]
        for t, x in (("qT", query), ("kT", key), ("vT", value))
    }
    in_maps = []
    for c in range(NCORE):
        b, g = divmod(c, GROUPS)
        f0 = g * FPC
        m = {
            "qT": xT["qT"][b],
            "kT": xT["kT"][b],
            "vT": xT["vT"][b],
            "wqT": np.ascontiguousarray(WqT[:, f0:f0 + FPC]).astype(BFNP),
            "wkT": np.ascontiguousarray(WkT[:, f0:f0 + FPC]).astype(BFNP),
            "wvT": np.ascontiguousarray(WvT[:, f0:f0 + FPC]).astype(BFNP),
            "woT": np.ascontiguousarray(WoT[f0:f0 + FPC, :]).astype(BFNP),
            "bqk": np.stack([bq[f0:f0 + FPC].reshape(FT, 128),
                             bk[f0:f0 + FPC].reshape(FT, 128)]).astype(np.float32),
        }
        if pat is not None:
            m["masks"] = pat
        in_maps.append(m)

    res = run_bass_kernel_spmd(
        nc, in_maps, core_ids=list(range(NCORE)),
        trace=PROFILE,
        trace_cores=(TRACE_CORES if TRACE_CORES is not None
                     else (list(range(NCORE)) if PROFILE else None)),
    )
    LAST_RESULT = res

    host_bias = bo + bv @ WoT  # (D,) folded V/O biases, added once per batch
    out = np.empty((B, S, D), np.float32)
    for b in range(B):
        acc = res.results[b * GROUPS]["out"].astype(np.float32)
        for g in range(1, GROUPS):
            acc = acc + res.results[b * GROUPS + g]["out"].astype(np.float32)
        out[b] = acc + host_bias
    return out


# revision 52
# speedup vs baseline: 1.0365x; 1.0134x over previous
"""Multi-head attention (B=2, S=2048, D=1024, H=16) on 8 Trainium2 NeuronCores.

Sharding: core c -> (batch b = c//4, head-group g = c%4).  Each core computes
Q/K/V projections for its 4 heads (256 features), causal attention for those
heads over the full sequence, and a partial O-projection (its 256 attn
features x full Wo.T slice).  The host sums the 4 partial outputs per batch
and folds in the biases that commute with the reduction (bo, bv @ Wo.T).

v2 structure (single fused pipeline, PE-dense):
  - Projections are split into per-chunk units (K/Q per 512-token chunk,
    V per 128-token tile, O per 128x512 block) and woven into the attention
    stream by a debt-based scheduler: the ScalarE exp stream (~82us, the
    attention-phase bottleneck) runs concurrently with projection matmuls
    instead of in a separate phase.
  - QK for the two heads of a feature tile are emitted adjacently; their
    lhsT base partitions (0/64) auto-derive row-tile positions T0/T8, so the
    two 64-contraction matmuls run concurrently in the PE array halves.
  - V is augmented with 64 columns of ones -> the PV matmul's output rows
    64..127 hold the softmax denominator replicated 64x, so normalization is
    two DVE ops (reciprocal_approx_fast + multiply); no partition broadcast.
  - exp on ScalarE with fused 1/sqrt(dk) scale; no max-subtraction (scores
    are O(5) for this data, exp is exact to 2 ULP, f32 cannot overflow).
  - masking: multiplicative bf16 tiles after exp, duplicated per head pair;
    partially-masked tiles carry a start column c0 so QK/exp/PV skip the
    dead q-range.
  - outputs are written bf16 (partials are summed f32 on host).
"""

import hashlib
from collections import deque
from contextlib import ExitStack

import ml_dtypes
import numpy as np

import concourse.bass as bass  # noqa: F401  (AP helpers)
import concourse.tile as tile
from concourse import bacc, mybir
from concourse.bass_utils import run_bass_kernel_spmd

B, S, D, H = 2, 2048, 1024, 16
DK = D // H                  # 64 head dim
NCORE = 8
GROUPS = NCORE // B          # 4 head-groups per batch
HPC = H // GROUPS            # 4 heads per core
FPC = HPC * DK               # 256 features per core
FT = FPC // 128              # 2 feature tiles per core
DT = D // 128                # 8 d_in tiles
TT = S // 128                # 16 token tiles (k tiles)
QB = 512                     # query block (free-dim) size in attention
NQB = S // QB                # 4 query blocks
NCH = 512                    # psum free-dim chunk for projections
XCH = 512                    # input stream DMA column chunk
VW = 2 * DK                  # augmented V width: [V | ones]
BF = mybir.dt.bfloat16
F32 = mybir.dt.float32
BFNP = ml_dtypes.bfloat16

# module-level knobs for test.py
PROFILE = False
TRACE_CORES = None
LAST_RESULT = None
DEBUG_TAPS = False

_program_cache: dict = {}


def _classify_mask(mask2d: np.ndarray):
    """Classify (S, S) keep-mask into per-(qblock, ktile) modes.

    Returns (plan, patterns): plan[qb] is a list of (kt, mask_id|None, c0, c1)
    for tiles that are at least partially kept, where c0 is the first
    q-column (within the block) with any kept key and [c0, c1) the strip
    needing the multiplicative mask; patterns is a list of [128, 2*w] bf16
    tiles (k on partitions, the mask strip duplicated for the head pair).
    """
    keep = np.asarray(mask2d) != 0
    patterns = []
    pattern_ids = {}
    plan = []
    for qb in range(NQB):
        row = []
        for kt in range(TT):
            blk = keep[qb * QB:(qb + 1) * QB, kt * 128:(kt + 1) * 128].T
            if not blk.any():
                continue
            if blk.all():
                row.append((kt, None, 0, 0))
                continue
            anyk = blk.any(axis=0)
            allk = blk.all(axis=0)
            c0 = int(np.flatnonzero(anyk)[0])
            notall = np.flatnonzero(~allk)
            c1 = int(notall[-1]) + 1 if notall.size else c0
            pat = blk[:, c0:c1]
            key = pat.tobytes()
            mid = pattern_ids.get(key)
            if mid is None:
                mid = len(patterns)
                pattern_ids[key] = mid
                # duplicate for the two heads sharing one [128, 2, w] mul
                patterns.append(np.concatenate([pat, pat], axis=1).astype(BFNP))
            row.append((kt, mid, c0, c1))
        plan.append(row)
    return plan, patterns


def build_program(plan, npat, pw):
    nc = bacc.Bacc("TRN2", target_bir_lowering=False, debug=False,
                   num_devices=NCORE)
    qT = nc.dram_tensor("qT", (D, S), BF, kind="ExternalInput").ap()
    kT = nc.dram_tensor("kT", (D, S), BF, kind="ExternalInput").ap()
    vT = nc.dram_tensor("vT", (D, S), BF, kind="ExternalInput").ap()
    wqT = nc.dram_tensor("wqT", (D, FPC), BF, kind="ExternalInput").ap()
    wkT = nc.dram_tensor("wkT", (D, FPC), BF, kind="ExternalInput").ap()
    wvT = nc.dram_tensor("wvT", (D, FPC), BF, kind="ExternalInput").ap()
    woT = nc.dram_tensor("woT", (FPC, D), BF, kind="ExternalInput").ap()
    bqk = nc.dram_tensor("bqk", (2, FT, 128), F32, kind="ExternalInput").ap()
    masks = None
    if npat:
        masks = nc.dram_tensor("masks", (npat, 128, pw), BF,
                               kind="ExternalInput").ap()
    out = nc.dram_tensor("out", (S, D), BF, kind="ExternalOutput").ap()
    taps = None
    if DEBUG_TAPS:
        taps = {
            n: nc.dram_tensor(f"dbg_{n}", shape, BF,
                              kind="ExternalOutput").ap()
            for n, shape in (("q", (128, FT, S)), ("k", (128, FT, S)),
                             ("attn", (128, FT, S)),
                             ("v", (128, TT, HPC, VW)))
        }

    with tile.TileContext(nc) as tc, ExitStack() as ctx:
        singles = ctx.enter_context(tc.tile_pool(name="singles", bufs=1))
        ppool = ctx.enter_context(tc.tile_pool(name="ppool", bufs=4))
        npool = ctx.enter_context(tc.tile_pool(name="npool", bufs=4))
        opool = ctx.enter_context(tc.tile_pool(name="opool", bufs=4))
        psacc = ctx.enter_context(tc.tile_pool(name="psacc", bufs=2, space="PSUM"))
        psS = ctx.enter_context(tc.tile_pool(name="psS", bufs=2, space="PSUM"))
        psPV = ctx.enter_context(tc.tile_pool(name="psPV", bufs=1, space="PSUM"))

        # ---- SBUF residents ----
        wq_sb = singles.tile([128, DT, FPC], BF)
        wk_sb = singles.tile([128, DT, FPC], BF)
        wv_sb = singles.tile([128, DT, FPC], BF)
        wo_sb = singles.tile([128, FT, D], BF)
        bias_sb = singles.tile([128, 2, FT], F32)
        mask_sb = None
        if npat:
            mask_sb = singles.tile([128, npat, pw], BF, name="mask_sb")
        q_sb = singles.tile([128, FT, S], BF)
        k_sb = singles.tile([128, FT, S], BF)
        attn_sb = singles.tile([128, FT, S], BF)
        v_sb = singles.tile([128, TT, HPC, VW], BF)
        xv_sb = singles.tile([128, DT, S], BF)
        xk_sb = singles.tile([128, DT, S], BF)
        xq_sb = singles.tile([128, DT, S], BF)

        # ---- DMA issue order: first-need first ----
        def stream_chunk(x_sb, x_dram, ch, split=False):
            xr = x_dram.rearrange("(t p) f -> p t f", p=128)
            if split:
                # per-d_in-tile descriptors so the first projection's
                # dt-loop can start before the whole chunk lands
                for dt in range(DT):
                    nc.sync.dma_start(
                        x_sb[:, dt, ch * XCH:(ch + 1) * XCH],
                        xr[:, dt, ch * XCH:(ch + 1) * XCH])
            else:
                nc.sync.dma_start(
                    x_sb[:, :, ch * XCH:(ch + 1) * XCH],
                    xr[:, :, ch * XCH:(ch + 1) * XCH])

        nc.sync.dma_start(wk_sb, wkT.rearrange("(t p) f -> p t f", p=128))
        stream_chunk(xk_sb, kT, 0, split=True)
        nc.sync.dma_start(bias_sb, bqk.rearrange("a b p -> p a b"))
        if npat:
            nc.sync.dma_start(mask_sb, masks.rearrange("m p f -> p m f"))
        nc.sync.dma_start(wq_sb, wqT.rearrange("(t p) f -> p t f", p=128))
        stream_chunk(xq_sb, qT, 0, split=True)
        nc.sync.dma_start(wv_sb, wvT.rearrange("(t p) f -> p t f", p=128))
        stream_chunk(xv_sb, vT, 0, split=True)
        nc.sync.dma_start(wo_sb, woT.rearrange("(t p) f -> p t f", p=128))
        for ch in range(1, S // XCH):
            stream_chunk(xk_sb, kT, ch)
            stream_chunk(xq_sb, qT, ch)
            stream_chunk(xv_sb, vT, ch)

        # ones columns of the augmented V (denominator accumulators).
        # Layout [ones | V]: the PV output then carries the denominator on
        # PSUM partitions 0..63 (reciprocal_approx_fast only works at base
        # partition 0) and attnU on partitions 64..127.
        for tt in range(TT):
            nc.vector.memset(v_sb[:, tt, :, 0:DK], 1.0)

        # ---- projection / output units (PE work the weave can draw on) ----
        # K/Q/V units are split into two half-groups (4 d_in tiles each) so
        # the weave can interleave them at sub-microsecond granularity; the
        # two halves share one PSUM accumulation tile via `held`.
        held = {}

        def kq_half(which, nch, ft, half):
            x_sb, w_sb = (xk_sb, wk_sb) if which == "k" else (xq_sb, wq_sb)
            if half == 0:
                held[(which, nch, ft)] = psacc.tile([128, NCH], F32,
                                                    tag="acc", name="acc")
            ps = held[(which, nch, ft)]
            for dt in range(4 * half, 4 * half + 4):
                nc.tensor.matmul(
                    ps,
                    lhsT=w_sb[:, dt, ft * 128:(ft + 1) * 128],
                    rhs=x_sb[:, dt, nch * NCH:(nch + 1) * NCH],
                    start=(dt == 0), stop=(dt == DT - 1))
            if half == 1:
                del held[(which, nch, ft)]
                cols = slice(nch * NCH, (nch + 1) * NCH)
                if which == "q":
                    nc.vector.tensor_scalar_add(
                        q_sb[:, ft, cols], ps, bias_sb[:, 0, ft:ft + 1])
                else:
                    nc.vector.tensor_scalar_add(
                        k_sb[:, ft, cols], ps, bias_sb[:, 1, ft:ft + 1])

        def v_half(tt, half):
            if half == 0:
                held[("v", tt)] = psacc.tile([128, FPC], F32,
                                             tag="acc", name="acc")
            ps = held[("v", tt)]
            for dt in range(4 * half, 4 * half + 4):
                nc.tensor.matmul(ps,
                                 lhsT=xv_sb[:, dt, tt * 128:(tt + 1) * 128],
                                 rhs=wv_sb[:, dt, :],
                                 start=(dt == 0), stop=(dt == DT - 1))
            if half == 1:
                del held[("v", tt)]
                nc.vector.tensor_copy(v_sb[:, tt, :, DK:VW],
                                      ps.rearrange("p (h d) -> p h d", h=HPC))

        def o_unit(qt, nch):
            ps = psacc.tile([128, NCH], F32, tag="acc")
            for hd in range(FT):
                nc.tensor.matmul(
                    ps,
                    lhsT=attn_sb[:, hd, qt * 128:(qt + 1) * 128],
                    rhs=wo_sb[:, hd, nch * NCH:(nch + 1) * NCH],
                    start=(hd == 0), stop=(hd == FT - 1))
            ob = opool.tile([128, NCH], BF)
            nc.vector.tensor_copy(ob, ps)
            nc.sync.dma_start(
                out[qt * 128:(qt + 1) * 128, nch * NCH:(nch + 1) * NCH], ob)

        # filler: (pe_ns, emit_fn, gate) — gate = phase index 2*qb+hp by
        # which the unit MUST have been emitted (its results feed that
        # phase's attention).
        KQ_NS = 4 * NCH / 2.4
        V_NS = 4 * FPC / 2.4
        O_NS = 2 * NCH / 2.4
        filler = deque()

        def push_proj(qb):
            # push in non-decreasing gate order (ft0+V at 2*qb, ft1 at
            # 2*qb+1) — the whole deque must stay gate-monotone
            for which in ("k", "q"):
                for half in (0, 1):
                    filler.append(
                        (KQ_NS,
                         lambda w=which, h=half: kq_half(w, qb, 0, h),
                         2 * qb))
            for tt in range(4 * qb, 4 * qb + 4):
                for half in (0, 1):
                    filler.append(
                        (V_NS, lambda t=tt, h=half: v_half(t, h), 2 * qb))
            for which in ("k", "q"):
                for half in (0, 1):
                    filler.append(
                        (KQ_NS,
                         lambda w=which, h=half: kq_half(w, qb, 1, h),
                         2 * qb + 1))

        def push_o(qb):
            for qt in range(qb * QB // 128, (qb + 1) * QB // 128):
                for nch in range(D // NCH):
                    filler.append(
                        (O_NS, lambda q=qt, n=nch: o_unit(q, n), 2 * qb + 4))

        state = {"debt": 0.0}

        def drain(cap=1):
            # emit at most `cap` filler units per call: consecutive filler
            # units would leave no attention matmuls between a projection
            # group's last matmul and the next group's first (which waits on
            # the DVE evacuation), exposing the cross-engine latency
            n_emitted = 0
            while state["debt"] > 0 and filler and n_emitted < cap:
                pe_ns, fn, gate = filler.popleft()
                fn()
                state["debt"] -= pe_ns
                n_emitted += 1

        def force(phase):
            # CORRECTNESS, not just pacing: every unit gated at or before
            # `phase` MUST be emitted before the attention that reads its
            # outputs, or that attention reads never-written memory (no
            # Tile dependency exists for a later-emitted producer).  Scan
            # the whole deque — do not trust front-of-queue gate order.
            rest = []
            while filler:
                item = filler.popleft()
                if item[2] <= phase:
                    item[1]()
                else:
                    rest.append(item)
            filler.extend(rest)

        # ---- prelude: minimum needed to start (qb0, hp0) attention ----
        for half in (0, 1):
            kq_half("k", 0, 0, half)
        for half in (0, 1):
            kq_half("q", 0, 0, half)
        for tt in (0, 1):
            for half in (0, 1):
                v_half(tt, half)
        # gate 0, NOT 1: qb0-hp0's PV for kt 2/3 reads these V tiles, and a
        # filler unit emitted after its consumer is a read-before-write race
        # (Tile has no dependency for never-yet-written memory)
        for tt in (2, 3):
            for half in (0, 1):
                filler.append((V_NS, lambda t=tt, h=half: v_half(t, h), 0))
        for which in ("k", "q"):
            for half in (0, 1):
                filler.append(
                    (KQ_NS, lambda w=which, h=half: kq_half(w, 0, 1, h), 1))

        # ---- fused attention pipeline ----
        inv_sqrt_dk = float(1.0 / np.sqrt(DK))
        for qb in range(NQB):
            if qb + 1 < NQB:
                push_proj(qb + 1)
            kts = plan[qb]
            if not kts:
                push_o(qb)
                continue
            n = len(kts)
            for hp in range(HPC // 2):
                force(2 * qb + hp)
                pv = [psPV.tile([128, QB], F32, tag=f"pv{j}", name=f"pv{j}")
                      for j in (0, 1)]

                def pv_mms(i, kt, c0, pT):
                    for j in (0, 1):
                        h = 2 * hp + j
                        nc.tensor.matmul(pv[j][:, c0:],
                                         lhsT=v_sb[:, kt, h, :],
                                         rhs=pT[:, j, c0:],
                                         start=(i == 0), stop=(i == n - 1))
                    state["debt"] -= 2 * (QB - c0) / 2.4

                def qk_mms(kt, c0):
                    s_ps = psS.tile([128, 2, QB], F32, name="s_ps")
                    for j in (0, 1):
                        pr = j * 64
                        nc.tensor.matmul(
                            s_ps[:, j, c0:],
                            lhsT=k_sb[pr:pr + DK, hp, kt * 128:(kt + 1) * 128],
                            rhs=q_sb[pr:pr + DK, hp,
                                     qb * QB + c0:(qb + 1) * QB],
                            start=True, stop=True)
                    state["debt"] -= (QB - c0) / 2.4
                    return s_ps

                # software pipeline: the exp for tile i is followed
                # immediately (in PE program order) by the QK of tile i+1,
                # so the ScalarE exp stream always has its next input ready;
                # filler and the PV of tile i-1 then absorb the remaining
                # exp window.
                prev = None
                s_cur = qk_mms(kts[0][0], kts[0][2])
                for i, (kt, mid, c0, c1) in enumerate(kts):
                    pT = ppool.tile([128, 2, QB], BF, tag="pt")
                    nc.scalar.activation(pT[:, :, c0:], s_cur[:, :, c0:],
                                         mybir.ActivationFunctionType.Exp,
                                         scale=inv_sqrt_dk)
                    if mid is not None and c1 > c0:
                        assert mask_sb is not None
                        w = c1 - c0
                        nc.vector.tensor_mul(
                            pT[:, :, c0:c1], pT[:, :, c0:c1],
                            mask_sb[:, mid, 0:2 * w].rearrange(
                                "p (j w) -> p j w", j=2))
                    if i + 1 < n:
                        s_cur = qk_mms(kts[i + 1][0], kts[i + 1][2])
                    state["debt"] += (2 * (QB - c0) + 360) / 1.2 + 250
                    drain()
                    if prev is not None:
                        pv_mms(*prev)
                    prev = (i, kt, c0, pT)
                if prev is not None:
                    pv_mms(*prev)
                # normalize: attn^T[d, q] = attnU^T[d, q] * (1/denom[q])
                for j in (0, 1):
                    pr = j * 64
                    rec = npool.tile([64, QB], F32, tag="rec")
                    nc.vector.reciprocal_approx_fast(rec, pv[j][0:DK, :])
                    nc.vector.tensor_mul(
                        attn_sb[pr:pr + DK, hp, qb * QB:(qb + 1) * QB],
                        pv[j][DK:VW, :], rec)
                # pull filler over the DVE normalize chain so the next
                # head-pair's first PV doesn't idle on the psPV banks
                state["debt"] += 1400
                drain(cap=2)
            push_o(qb)

        # drain what's left (the last O-projection blocks mostly)
        while filler:
            pe_ns, fn, gate = filler.popleft()
            fn()

        if taps is not None:
            nc.sync.dma_start(taps["q"], q_sb)
            nc.sync.dma_start(taps["k"], k_sb)
            nc.sync.dma_start(taps["attn"], attn_sb)
            nc.sync.dma_start(taps["v"], v_sb)

    nc.compile()
    return nc


def _get_program(mask2d: np.ndarray):
    key = (hashlib.sha1(np.ascontiguousarray(mask2d).tobytes()).hexdigest()
           + ("+taps" if DEBUG_TAPS else ""))
    hit = _program_cache.get(key)
    if hit is not None:
        return hit
    plan, patterns = _classify_mask(mask2d)
    pw = max((p.shape[1] for p in patterns), default=0)
    nc = build_program(plan, len(patterns), pw)
    if patterns:
        pat = np.zeros((len(patterns), 128, pw), BFNP)
        for i, p in enumerate(patterns):
            pat[i, :, :p.shape[1]] = p
    else:
        pat = None
    _program_cache[key] = (nc, pat)
    return nc, pat


def kernel(**inputs) -> np.ndarray:
    global LAST_RESULT
    query = np.asarray(inputs["query"], np.float32)
    key = np.asarray(inputs["key"], np.float32)
    value = np.asarray(inputs["value"], np.float32)
    mask = np.asarray(inputs["mask"])
    Wq = np.asarray(inputs["Wq"], np.float32)
    bq = np.asarray(inputs["bq"], np.float32)
    Wk = np.asarray(inputs["Wk"], np.float32)
    bk = np.asarray(inputs["bk"], np.float32)
    Wv = np.asarray(inputs["Wv"], np.float32)
    bv = np.asarray(inputs["bv"], np.float32)
    Wo = np.asarray(inputs["Wo"], np.float32)
    bo = np.asarray(inputs["bo"], np.float32)

    nc, pat = _get_program(mask.reshape(S, S))

    WqT, WkT, WvT, WoT = Wq.T, Wk.T, Wv.T, Wo.T
    xT = {
        t: [np.ascontiguousarray(x[b].T).astype(BFNP) for b in range(B)]
        for t, x in (("qT", query), ("kT", key), ("vT", value))
    }
    in_maps = []
    for c in range(NCORE):
        b, g = divmod(c, GROUPS)
        f0 = g * FPC
        m = {
            "qT": xT["qT"][b],
            "kT": xT["kT"][b],
            "vT": xT["vT"][b],
            "wqT": np.ascontiguousarray(WqT[:, f0:f0 + FPC]).astype(BFNP),
            "wkT": np.ascontiguousarray(WkT[:, f0:f0 + FPC]).astype(BFNP),
            "wvT": np.ascontiguousarray(WvT[:, f0:f0 + FPC]).astype(BFNP),
            "woT": np.ascontiguousarray(WoT[f0:f0 + FPC, :]).astype(BFNP),
            "bqk": np.stack([bq[f0:f0 + FPC].reshape(FT, 128),
                             bk[f0:f0 + FPC].reshape(FT, 128)]).astype(np.float32),
        }
        if pat is not None:
            m["masks"] = pat
        in_maps.append(m)

    res = run_bass_kernel_spmd(
        nc, in_maps, core_ids=list(range(NCORE)),
        trace=PROFILE,
        trace_cores=(TRACE_CORES if TRACE_CORES is not None
                     else (list(range(NCORE)) if PROFILE else None)),
    )
    LAST_RESULT = res

    host_bias = bo + bv @ WoT  # (D,) folded V/O biases, added once per batch
    out = np.empty((B, S, D), np.float32)
    for b in range(B):
        acc = res.results[b * GROUPS]["out"].astype(np.float32)
        for g in range(1, GROUPS):
            acc = acc + res.results[b * GROUPS + g]["out"].astype(np.float32)
        out[b] = acc + host_bias
    return out
